# revision 14
# baseline (speedup 1.0000x reference)
"""Trainium2 Bass kernel for nn_RandomFlip_45999099740646.

Reference semantics (vanilla jax, threefry PRNG):
    prop = jax.random.uniform(jax.random.key(42), (), float32) = 0.48870957
    prop <= RATE (0.5)  ->  the horizontal flip IS applied:
        images_out[n, h, w, c] = images[n, h, W-1-w, c]
        boxes_out = boxes * [-1, 1, -1, 1] + [640, 0, 640, 0]
        distances, classes unchanged.

Sharding: pure data parallel over the batch dim; 4 images (+ their 200
boxes) per NeuronCore, 8 cores, no communication.

Per-core images shard is viewed as a [2560, 1920] f32 matrix (row = one
image row, 640 px * 3 ch).  Raw-bass pipeline, per [128, 1920] tile:
    SP   (HWDGE)  DMA in, contiguous 7680B chunk per partition
    DVE           reversed-stride tensor_copy (w axis, stride -3; 2x mode)
    ACT  (HWDGE)  DMA out, contiguous
10-deep buffering, one DMA-completion semaphore lane per buffer slot.
Measured: gapless DMA stream at ~358 GB/s per core (the HBM-per-NC
limit); typical core exec ~105 us vs the 109.8 us 358-GB/s floor.
Boxes are transformed with a single tensor_scalar (x*-1 + 640) on the
x columns and a copy on the y columns.
"""

import contextlib
import sys
import types

import numpy as np

import concourse.bacc as bacc
import concourse.mybir as mybir
from concourse.tile import TileContext
from concourse.bass_utils import run_bass_kernel_spmd

N, H, W, C = 32, 640, 640, 3
NCORES = 8
NPER = N // NCORES            # images per core
ROWS = NPER * H               # 2560 image rows per core
F = W * C                     # 1920 floats per image row
P = 128                       # SBUF partitions
G = ROWS // P                 # 20 row-groups per core
KG = 1                        # row-groups per tile -> 20 tiles of ~983 KB
BUFS = 10                     # pipeline depth (2 pools x 10 x 7.5KB/part)
NBOX = N * 50 // NCORES       # 200 boxes per core
BROWS = NBOX // 2             # boxes viewed as [100, 8]
IMG_SIZE = 640.0

_cache = {}

# If the caller's environment sets BASS_TRACE, run_bass_kernel_spmd tries
# `from antenv.axon_hooks import get_axon_ntff_profile_hook`, a module the
# agent image does not ship.  Register a benign stub so tracing degrades
# gracefully instead of raising.
try:
    import antenv.axon_hooks  # noqa: F401
except ImportError:
    try:
        import antenv  # noqa: F401
        _m = types.ModuleType("antenv.axon_hooks")
        _m.get_axon_ntff_profile_hook = lambda: None
        _m.set_axon_ntff_profile_hook = lambda h: None
        sys.modules["antenv.axon_hooks"] = _m
    except ImportError:
        pass


def _build(rows=ROWS, kg=KG, brows=BROWS):
    """Build the Bass module (same program runs SPMD on every core)."""
    f32 = mybir.dt.float32
    nc = bacc.Bacc()
    img_in = nc.dram_tensor("img_in", (rows, F), f32, kind="ExternalInput")
    box_in = nc.dram_tensor("box_in", (brows, 8), f32, kind="ExternalInput")
    img_out = nc.dram_tensor("img_out", (rows, F), f32, kind="ExternalOutput")
    box_out = nc.dram_tensor("box_out", (brows, 8), f32, kind="ExternalOutput")

    g = rows // P
    ntiles = g // kg
    assert g % kg == 0

    # row (128*gi + p) <-> partition p, free (gi, f); contiguous per chunk
    vin = img_in.rearrange("(g p) f -> p g f", p=P)
    vout = img_out.rearrange("(g p) f -> p g f", p=P)

    with TileContext(nc) as tc:
        with (
            tc.tile_pool(name="bx", bufs=1) as bxp,
            tc.tile_pool(name="tin", bufs=4) as pin,
            tc.tile_pool(name="tout", bufs=4) as pout,
        ):
            # ---- images: stream tiles, reverse w on-chip ----
            for t in range(ntiles):
                ti = pin.tile([P, kg * F], f32)
                to = pout.tile([P, kg * F], f32)
                nc.sync.dma_start(ti[:, :], vin[:, t * kg:(t + 1) * kg, :])
                seg_i = ti[:, :].rearrange("p (g w c) -> p g w c", g=kg, c=C)
                seg_o = to[:, :].rearrange("p (g w c) -> p g w c", g=kg, c=C)
                nc.vector.tensor_copy(seg_o, seg_i[:, :, ::-1, :])
                nc.sync.dma_start(vout[:, t * kg:(t + 1) * kg, :], to[:, :])
                if t == 0:
                    # boxes ride behind the first image tile: gpsimd DMAs +
                    # scalar-engine math keep them off the image stream
                    bi = bxp.tile([brows, 8], f32)
                    bo = bxp.tile([brows, 8], f32)
                    nc.gpsimd.dma_start(bi[:, :], box_in[:, :])
                    nc.scalar.activation(
                        bo[:, 0:8:2], bi[:, 0:8:2],
                        mybir.ActivationFunctionType.Copy,
                        bias=IMG_SIZE, scale=-1.0)
                    nc.scalar.copy(bo[:, 1:8:2], bi[:, 1:8:2])
                    nc.gpsimd.dma_start(box_out[:, :], bo[:, :])
    nc.finalize()
    return nc


def _build_raw(rows=ROWS, kg=KG, brows=BROWS, bufs=BUFS):
    """Raw-bass pipeline (no TileContext): explicit semaphores, no
    start/exit all-engine barriers.  SP issues in-DMAs, DVE reverses,
    ACT issues out-DMAs; boxes ride the gpsimd queue + DVE."""
    f32 = mybir.dt.float32
    nc = bacc.Bacc()
    img_in = nc.dram_tensor("img_in", (rows, F), f32, kind="ExternalInput")
    box_in = nc.dram_tensor("box_in", (brows, 8), f32, kind="ExternalInput")
    img_out = nc.dram_tensor("img_out", (rows, F), f32, kind="ExternalOutput")
    box_out = nc.dram_tensor("box_out", (brows, 8), f32, kind="ExternalOutput")

    g = rows // P
    ntiles = g // kg
    assert g % kg == 0
    seg = kg * F

    vin = img_in.rearrange("(g p) f -> p g f", p=P)
    vout = img_out.rearrange("(g p) f -> p g f", p=P)

    with contextlib.ExitStack() as ctx:
        ti = ctx.enter_context(nc.sbuf_tensor("ti", [P, bufs * seg], f32))
        to = ctx.enter_context(nc.sbuf_tensor("to", [P, bufs * seg], f32))
        bi = ctx.enter_context(nc.sbuf_tensor("bi", [brows, 8], f32))
        bo = ctx.enter_context(nc.sbuf_tensor("bo", [brows, 8], f32))
        # one DMA-completion sem lane per buffer slot: concurrent DMAs on a
        # shared sem would interleave their 16 per-engine +1s (racy)
        s_in = [ctx.enter_context(nc.semaphore(f"s_in{b}"))
                for b in range(bufs)]
        s_out = [ctx.enter_context(nc.semaphore(f"s_out{b}"))
                 for b in range(bufs)]
        s_rev = ctx.enter_context(nc.semaphore("s_rev"))
        s_bx = ctx.enter_context(nc.semaphore("s_bx"))
        s_bxd = ctx.enter_context(nc.semaphore("s_bxd"))
        s_bout = ctx.enter_context(nc.semaphore("s_bout"))

        # ---- boxes: gpsimd DMA in -> DVE math -> gpsimd DMA out ----
        nc.gpsimd.dma_start(bi[:, :], box_in[:, :]).then_inc(s_bx, 16)
        nc.vector.wait_ge(s_bx, 16)
        nc.vector.tensor_scalar(
            out=bo[:, 0:8:2], in0=bi[:, 0:8:2],
            scalar1=-1.0, scalar2=IMG_SIZE,
            op0=mybir.AluOpType.mult, op1=mybir.AluOpType.add)
        nc.vector.tensor_copy(bo[:, 1:8:2], bi[:, 1:8:2]).then_inc(s_bxd, 1)
        nc.gpsimd.wait_ge(s_bxd, 1)
        nc.gpsimd.dma_start(box_out[:, :], bo[:, :]).then_inc(s_bout, 16)
        nc.gpsimd.wait_ge(s_bout, 16)

        # ---- image pipeline ----
        for t in range(ntiles):
            b = t % bufs
            k = t // bufs          # how many times slot b has been used
            sl = b * seg
            # SP: in-DMA (slot free once copy t-bufs retired it)
            if t >= bufs:
                nc.sync.wait_ge(s_rev, t - bufs + 1)
            nc.sync.dma_start(
                ti[:, sl:sl + seg], vin[:, t * kg:(t + 1) * kg, :]
            ).then_inc(s_in[b], 16)
            # DVE: reversal copy
            nc.vector.wait_ge(s_in[b], 16 * (k + 1))
            if t >= bufs:
                nc.vector.wait_ge(s_out[b], 16 * k)
            seg_i = ti[:, sl:sl + seg].rearrange(
                "p (g w c) -> p g w c", g=kg, c=C)
            seg_o = to[:, sl:sl + seg].rearrange(
                "p (g w c) -> p g w c", g=kg, c=C)
            nc.vector.tensor_copy(
                seg_o, seg_i[:, :, ::-1, :]).then_inc(s_rev, 1)
            # ACT: out-DMA
            nc.scalar.wait_ge(s_rev, t + 1)
            nc.scalar.dma_start(
                vout[:, t * kg:(t + 1) * kg, :], to[:, sl:sl + seg]
            ).then_inc(s_out[b], 16)
        # make sure every output byte has landed before the NEFF ends
        for b in range(bufs):
            uses = (ntiles - b + bufs - 1) // bufs
            if uses:
                nc.scalar.wait_ge(s_out[b], 16 * uses)
    nc.finalize()
    return nc


def kernel(images, boxes, distances, classes):
    images = np.ascontiguousarray(np.asarray(images, dtype=np.float32))
    boxes = np.ascontiguousarray(np.asarray(boxes, dtype=np.float32))
    distances = np.asarray(distances)
    classes = np.asarray(classes)

    if "nc" not in _cache:
        _cache["nc"] = _build_raw()
    nc = _cache["nc"]

    img_sh = images.reshape(NCORES, ROWS, F)
    box_sh = boxes.reshape(NCORES, BROWS, 8)
    in_maps = [
        {"img_in": img_sh[c], "box_in": box_sh[c]} for c in range(NCORES)
    ]
    res = run_bass_kernel_spmd(nc, in_maps, core_ids=list(range(NCORES)))
    _cache["last_results"] = res

    imgs = np.concatenate(
        [res.results[c]["img_out"].reshape(NPER, H, W, C)
         for c in range(NCORES)], axis=0)
    bxs = np.concatenate(
        [res.results[c]["box_out"].reshape(NPER, 50, 4)
         for c in range(NCORES)], axis=0)
    return imgs, bxs, distances, classes


# revision 21
# speedup vs baseline: 1.0472x; 1.0472x over previous
"""Trainium2 Bass kernel for nn_RandomFlip_45999099740646.

Reference semantics (vanilla jax, threefry PRNG):
    prop = jax.random.uniform(jax.random.key(42), (), float32) = 0.48870957
    prop <= RATE (0.5)  ->  the horizontal flip IS applied:
        images_out[n, h, w, c] = images[n, h, W-1-w, c]
        boxes_out = boxes * [-1, 1, -1, 1] + [640, 0, 640, 0]
        distances, classes unchanged.

Sharding: pure data parallel over the batch dim; 4 images (+ their 200
boxes) per NeuronCore, 8 cores, no communication.

Per-core images shard is viewed as a [2560, 1920] f32 matrix (row = one
image row, 640 px * 3 ch).  Raw-bass pipeline, per [128, 1920] tile:
    SP   (HWDGE)  DMA in, contiguous 7680B chunk per partition
    DVE           reversed-stride tensor_copy (w axis, stride -3; 2x mode)
    ACT  (HWDGE)  DMA out, contiguous
10-deep buffering, one DMA-completion semaphore lane per buffer slot.
Measured: gapless DMA stream at ~358 GB/s per core (the HBM-per-NC
limit); typical core exec ~105 us vs the 109.8 us 358-GB/s floor.
Boxes are transformed with a single tensor_scalar (x*-1 + 640) on the
x columns and a copy on the y columns.
"""

import contextlib
import sys
import types

import numpy as np

import concourse.bacc as bacc
import concourse.mybir as mybir
from concourse.tile import TileContext
from concourse.bass_utils import run_bass_kernel_spmd

N, H, W, C = 32, 640, 640, 3
NCORES = 8
NPER = N // NCORES            # images per core
ROWS = NPER * H               # 2560 image rows per core
F = W * C                     # 1920 floats per image row
P = 128                       # SBUF partitions
G = ROWS // P                 # 20 row-groups per core
KG = 1                        # row-groups per tile -> 20 tiles of ~983 KB
BUFS = 10                     # pipeline depth (2 pools x 10 x 7.5KB/part)
NBOX = N * 50 // NCORES       # 200 boxes per core
BROWS = NBOX // 2             # boxes viewed as [100, 8]
IMG_SIZE = 640.0

_cache = {}

# If the caller's environment sets BASS_TRACE, run_bass_kernel_spmd tries
# `from antenv.axon_hooks import get_axon_ntff_profile_hook`, a module the
# agent image does not ship.  Register a benign stub so tracing degrades
# gracefully instead of raising.
try:
    import antenv.axon_hooks  # noqa: F401
except ImportError:
    try:
        import antenv  # noqa: F401
        _m = types.ModuleType("antenv.axon_hooks")
        _m.get_axon_ntff_profile_hook = lambda: None
        _m.set_axon_ntff_profile_hook = lambda h: None
        sys.modules["antenv.axon_hooks"] = _m
    except ImportError:
        pass


def _build(rows=ROWS, kg=KG, brows=BROWS):
    """Build the Bass module (same program runs SPMD on every core)."""
    f32 = mybir.dt.float32
    nc = bacc.Bacc()
    img_in = nc.dram_tensor("img_in", (rows, F), f32, kind="ExternalInput")
    box_in = nc.dram_tensor("box_in", (brows, 8), f32, kind="ExternalInput")
    img_out = nc.dram_tensor("img_out", (rows, F), f32, kind="ExternalOutput")
    box_out = nc.dram_tensor("box_out", (brows, 8), f32, kind="ExternalOutput")

    g = rows // P
    ntiles = g // kg
    assert g % kg == 0

    # row (128*gi + p) <-> partition p, free (gi, f); contiguous per chunk
    vin = img_in.rearrange("(g p) f -> p g f", p=P)
    vout = img_out.rearrange("(g p) f -> p g f", p=P)

    with TileContext(nc) as tc:
        with (
            tc.tile_pool(name="bx", bufs=1) as bxp,
            tc.tile_pool(name="tin", bufs=4) as pin,
            tc.tile_pool(name="tout", bufs=4) as pout,
        ):
            # ---- images: stream tiles, reverse w on-chip ----
            for t in range(ntiles):
                ti = pin.tile([P, kg * F], f32)
                to = pout.tile([P, kg * F], f32)
                nc.sync.dma_start(ti[:, :], vin[:, t * kg:(t + 1) * kg, :])
                seg_i = ti[:, :].rearrange("p (g w c) -> p g w c", g=kg, c=C)
                seg_o = to[:, :].rearrange("p (g w c) -> p g w c", g=kg, c=C)
                nc.vector.tensor_copy(seg_o, seg_i[:, :, ::-1, :])
                nc.sync.dma_start(vout[:, t * kg:(t + 1) * kg, :], to[:, :])
                if t == 0:
                    # boxes ride behind the first image tile: gpsimd DMAs +
                    # scalar-engine math keep them off the image stream
                    bi = bxp.tile([brows, 8], f32)
                    bo = bxp.tile([brows, 8], f32)
                    nc.gpsimd.dma_start(bi[:, :], box_in[:, :])
                    nc.scalar.activation(
                        bo[:, 0:8:2], bi[:, 0:8:2],
                        mybir.ActivationFunctionType.Copy,
                        bias=IMG_SIZE, scale=-1.0)
                    nc.scalar.copy(bo[:, 1:8:2], bi[:, 1:8:2])
                    nc.gpsimd.dma_start(box_out[:, :], bo[:, :])
    nc.finalize()
    return nc


def _build_raw(rows=ROWS, kg=KG, brows=BROWS, bufs=BUFS):
    """Raw-bass pipeline (no TileContext): explicit semaphores, no
    start/exit all-engine barriers.  SP issues in-DMAs, DVE reverses,
    ACT issues out-DMAs; boxes ride the gpsimd queue + DVE."""
    import concourse.bass as bass_mod
    f32 = mybir.dt.float32
    # Suppress the Bass.__init__ preamble: 4 const-AP memsets on GpSimd
    # (this kernel never reads a const AP) and the all-engine barrier that
    # makes every engine wait for them (~3-5us at kernel start; all
    # cross-engine deps here are explicit semaphores).  Fall back to the
    # stock preamble if the bass internals ever change shape.
    _patch_ok = (hasattr(bass_mod, "BassGpSimd")
                 and "memset" not in bass_mod.BassGpSimd.__dict__
                 and hasattr(bass_mod.Bass, "all_engine_barrier"))
    _orig_barrier = getattr(bass_mod.Bass, "all_engine_barrier", None)
    try:
        if _patch_ok:
            bass_mod.BassGpSimd.memset = lambda self, ap, c: None
            bass_mod.Bass.all_engine_barrier = (
                lambda self, *, sem_only=False: None)
        nc = bacc.Bacc()
    finally:
        if _patch_ok:
            del bass_mod.BassGpSimd.memset
            bass_mod.Bass.all_engine_barrier = _orig_barrier
    img_in = nc.dram_tensor("img_in", (rows, F), f32, kind="ExternalInput")
    box_in = nc.dram_tensor("box_in", (brows, 8), f32, kind="ExternalInput")
    img_out = nc.dram_tensor("img_out", (rows, F), f32, kind="ExternalOutput")
    box_out = nc.dram_tensor("box_out", (brows, 8), f32, kind="ExternalOutput")

    g = rows // P
    ntiles = g // kg
    assert g % kg == 0
    seg = kg * F

    vin = img_in.rearrange("(g p) f -> p g f", p=P)
    vout = img_out.rearrange("(g p) f -> p g f", p=P)

    with contextlib.ExitStack() as ctx:
        ti = ctx.enter_context(nc.sbuf_tensor("ti", [P, bufs * seg], f32))
        to = ctx.enter_context(nc.sbuf_tensor("to", [P, bufs * seg], f32))
        bi = ctx.enter_context(nc.sbuf_tensor("bi", [brows, 8], f32))
        bo = ctx.enter_context(nc.sbuf_tensor("bo", [brows, 8], f32))
        # one DMA-completion sem lane per buffer slot: concurrent DMAs on a
        # shared sem would interleave their 16 per-engine +1s (racy)
        s_in = [ctx.enter_context(nc.semaphore(f"s_in{b}"))
                for b in range(bufs)]
        s_out = [ctx.enter_context(nc.semaphore(f"s_out{b}"))
                 for b in range(bufs)]
        s_rev = ctx.enter_context(nc.semaphore("s_rev"))
        s_bx = ctx.enter_context(nc.semaphore("s_bx"))
        s_bxd = ctx.enter_context(nc.semaphore("s_bxd"))
        s_bout = ctx.enter_context(nc.semaphore("s_bout"))

        # ---- boxes: ACT DMA in -> DVE math -> ACT DMA out.  The ACT
        # (out) ring is idle at kernel start, so the box transfer never
        # delays the image in-stream on SP; no gpsimd / PE instructions
        # anywhere ----
        nc.scalar.dma_start(bi[:, :], box_in[:, :]).then_inc(s_bx, 16)
        nc.vector.wait_ge(s_bx, 16)
        nc.vector.tensor_scalar(
            out=bo[:, 0:8:2], in0=bi[:, 0:8:2],
            scalar1=-1.0, scalar2=IMG_SIZE,
            op0=mybir.AluOpType.mult, op1=mybir.AluOpType.add)
        nc.vector.tensor_copy(bo[:, 1:8:2], bi[:, 1:8:2]).then_inc(s_bxd, 1)
        nc.scalar.wait_ge(s_bxd, 1)
        nc.scalar.dma_start(box_out[:, :], bo[:, :]).then_inc(s_bout, 16)

        # ---- image pipeline ----
        # Tile specs (row-group, col0, col1).  The last two row-groups are
        # split into half-rows: after the final in-DMA byte the stream is
        # extended only by copy+out of the final tile, so smaller final
        # tiles shorten the pipeline drain chain.
        assert kg == 1 and ntiles >= 4
        half = (F // 2) // C * C       # 960, pixel-aligned
        quart = (F // 4) // C * C      # 480
        specs = [(g, 0, F) for g in range(ntiles - 2)]
        g = ntiles - 2
        specs += [(g, 0, half), (g, half, F)]
        g = ntiles - 1
        specs += [(g, q * quart, (q + 1) * quart) for q in range(4)]

        for t, (g, f0, f1) in enumerate(specs):
            b = t % bufs
            k = t // bufs          # how many times slot b has been used
            sl = b * seg
            w = f1 - f0
            # SP: in-DMA (slot free once copy t-bufs retired it)
            if t >= bufs:
                nc.sync.wait_ge(s_rev, t - bufs + 1)
            nc.sync.dma_start(
                ti[:, sl:sl + w], vin[:, g, f0:f1]
            ).then_inc(s_in[b], 16)
            # DVE: reversal copy (piece reversed internally; its mirrored
            # position within the row is handled by the out-DMA offset)
            nc.vector.wait_ge(s_in[b], 16 * (k + 1))
            if t >= bufs:
                nc.vector.wait_ge(s_out[b], 16 * k)
            seg_i = ti[:, sl:sl + w].rearrange("p (w c) -> p w c", c=C)
            seg_o = to[:, sl:sl + w].rearrange("p (w c) -> p w c", c=C)
            nc.vector.tensor_copy(
                seg_o, seg_i[:, ::-1, :]).then_inc(s_rev, 1)
            # ACT: out-DMA into the mirrored column range
            nc.scalar.wait_ge(s_rev, t + 1)
            nc.scalar.dma_start(
                vout[:, g, F - f1:F - f0], to[:, sl:sl + w]
            ).then_inc(s_out[b], 16)
        # make sure every output byte has landed before the NEFF ends
        nc.scalar.wait_ge(s_bout, 16)
        ntot = len(specs)
        for b in range(bufs):
            uses = (ntot - b + bufs - 1) // bufs
            if uses:
                nc.scalar.wait_ge(s_out[b], 16 * uses)
    nc.finalize()
    return nc


def kernel(images, boxes, distances, classes):
    images = np.ascontiguousarray(np.asarray(images, dtype=np.float32))
    boxes = np.ascontiguousarray(np.asarray(boxes, dtype=np.float32))
    distances = np.asarray(distances)
    classes = np.asarray(classes)

    if "nc" not in _cache:
        _cache["nc"] = _build_raw()
    nc = _cache["nc"]

    img_sh = images.reshape(NCORES, ROWS, F)
    box_sh = boxes.reshape(NCORES, BROWS, 8)
    in_maps = [
        {"img_in": img_sh[c], "box_in": box_sh[c]} for c in range(NCORES)
    ]
    res = run_bass_kernel_spmd(nc, in_maps, core_ids=list(range(NCORES)))
    _cache["last_results"] = res

    imgs = np.concatenate(
        [res.results[c]["img_out"].reshape(NPER, H, W, C)
         for c in range(NCORES)], axis=0)
    bxs = np.concatenate(
        [res.results[c]["box_out"].reshape(NPER, 50, 4)
         for c in range(NCORES)], axis=0)
    return imgs, bxs, distances, classes


# revision 25
# speedup vs baseline: 1.1526x; 1.1007x over previous
"""Trainium2 Bass kernel for nn_RandomFlip_45999099740646.

Reference semantics (vanilla jax, threefry PRNG):
    prop = jax.random.uniform(jax.random.key(42), (), float32) = 0.48870957
    prop <= RATE (0.5)  ->  the horizontal flip IS applied:
        images_out[n, h, w, c] = images[n, h, W-1-w, c]
        boxes_out = boxes * [-1, 1, -1, 1] + [640, 0, 640, 0]
        distances, classes unchanged.

Sharding: pure data parallel over the batch dim; 4 images (+ their 200
boxes) per NeuronCore, 8 cores, no communication.

Per-core images shard is viewed as a [2560, 1920] f32 matrix (row = one
image row, 640 px * 3 ch).  Raw-bass pipeline, per [128, 1920] tile:
    SP   (HWDGE)  DMA in, contiguous 7680B chunk per partition
    DVE           reversed-stride tensor_copy (w axis, stride -3; 2x mode)
    ACT  (HWDGE)  DMA out, contiguous
10-deep buffering, one DMA-completion semaphore lane per buffer slot.
Measured: gapless DMA stream at ~358 GB/s per core (the HBM-per-NC
limit); typical core exec ~105 us vs the 109.8 us 358-GB/s floor.
Boxes are transformed with a single tensor_scalar (x*-1 + 640) on the
x columns and a copy on the y columns.
"""

import contextlib
import sys
import types

import numpy as np

import concourse.bacc as bacc
import concourse.mybir as mybir
from concourse.tile import TileContext
from concourse.bass_utils import run_bass_kernel_spmd

N, H, W, C = 32, 640, 640, 3
NCORES = 8
NPER = N // NCORES            # images per core
ROWS = NPER * H               # 2560 image rows per core
F = W * C                     # 1920 floats per image row
P = 128                       # SBUF partitions
G = ROWS // P                 # 20 row-groups per core
KG = 1                        # row-groups per tile -> 20 tiles of ~983 KB
BUFS = 10                     # pipeline depth (2 pools x 10 x 7.5KB/part)
NBOX = N * 50 // NCORES       # 200 boxes per core
BROWS = NBOX // 2             # boxes viewed as [100, 8]
IMG_SIZE = 640.0

_cache = {}

# If the caller's environment sets BASS_TRACE, run_bass_kernel_spmd tries
# `from antenv.axon_hooks import get_axon_ntff_profile_hook`, a module the
# agent image does not ship.  Register a benign stub so tracing degrades
# gracefully instead of raising.
try:
    import antenv.axon_hooks  # noqa: F401
except ImportError:
    try:
        import antenv  # noqa: F401
        _m = types.ModuleType("antenv.axon_hooks")
        _m.get_axon_ntff_profile_hook = lambda: None
        _m.set_axon_ntff_profile_hook = lambda h: None
        sys.modules["antenv.axon_hooks"] = _m
    except ImportError:
        pass


def _build(rows=ROWS, kg=KG, brows=BROWS):
    """Build the Bass module (same program runs SPMD on every core)."""
    f32 = mybir.dt.float32
    nc = bacc.Bacc()
    img_in = nc.dram_tensor("img_in", (rows, F), f32, kind="ExternalInput")
    box_in = nc.dram_tensor("box_in", (brows, 8), f32, kind="ExternalInput")
    img_out = nc.dram_tensor("img_out", (rows, F), f32, kind="ExternalOutput")
    box_out = nc.dram_tensor("box_out", (brows, 8), f32, kind="ExternalOutput")

    g = rows // P
    ntiles = g // kg
    assert g % kg == 0

    # row (128*gi + p) <-> partition p, free (gi, f); contiguous per chunk
    vin = img_in.rearrange("(g p) f -> p g f", p=P)
    vout = img_out.rearrange("(g p) f -> p g f", p=P)

    with TileContext(nc) as tc:
        with (
            tc.tile_pool(name="bx", bufs=1) as bxp,
            tc.tile_pool(name="tin", bufs=4) as pin,
            tc.tile_pool(name="tout", bufs=4) as pout,
        ):
            # ---- images: stream tiles, reverse w on-chip ----
            for t in range(ntiles):
                ti = pin.tile([P, kg * F], f32)
                to = pout.tile([P, kg * F], f32)
                nc.sync.dma_start(ti[:, :], vin[:, t * kg:(t + 1) * kg, :])
                seg_i = ti[:, :].rearrange("p (g w c) -> p g w c", g=kg, c=C)
                seg_o = to[:, :].rearrange("p (g w c) -> p g w c", g=kg, c=C)
                nc.vector.tensor_copy(seg_o, seg_i[:, :, ::-1, :])
                nc.sync.dma_start(vout[:, t * kg:(t + 1) * kg, :], to[:, :])
                if t == 0:
                    # boxes ride behind the first image tile: gpsimd DMAs +
                    # scalar-engine math keep them off the image stream
                    bi = bxp.tile([brows, 8], f32)
                    bo = bxp.tile([brows, 8], f32)
                    nc.gpsimd.dma_start(bi[:, :], box_in[:, :])
                    nc.scalar.activation(
                        bo[:, 0:8:2], bi[:, 0:8:2],
                        mybir.ActivationFunctionType.Copy,
                        bias=IMG_SIZE, scale=-1.0)
                    nc.scalar.copy(bo[:, 1:8:2], bi[:, 1:8:2])
                    nc.gpsimd.dma_start(box_out[:, :], bo[:, :])
    nc.finalize()
    return nc


def _build_raw(rows=ROWS, kg=KG, brows=BROWS, bufs=BUFS):
    """Raw-bass pipeline (no TileContext): explicit semaphores, no
    start/exit all-engine barriers.  SP issues in-DMAs, DVE reverses,
    ACT issues out-DMAs; boxes ride the gpsimd queue + DVE."""
    import concourse.bass as bass_mod
    f32 = mybir.dt.float32
    # Suppress the Bass.__init__ preamble: 4 const-AP memsets on GpSimd
    # (this kernel never reads a const AP) and the all-engine barrier that
    # makes every engine wait for them (~3-5us at kernel start; all
    # cross-engine deps here are explicit semaphores).  Fall back to the
    # stock preamble if the bass internals ever change shape.
    _patch_ok = (hasattr(bass_mod, "BassGpSimd")
                 and "memset" not in bass_mod.BassGpSimd.__dict__
                 and hasattr(bass_mod.Bass, "all_engine_barrier"))
    _orig_barrier = getattr(bass_mod.Bass, "all_engine_barrier", None)
    try:
        if _patch_ok:
            bass_mod.BassGpSimd.memset = lambda self, ap, c: None
            bass_mod.Bass.all_engine_barrier = (
                lambda self, *, sem_only=False: None)
        nc = bacc.Bacc()
    finally:
        if _patch_ok:
            del bass_mod.BassGpSimd.memset
            bass_mod.Bass.all_engine_barrier = _orig_barrier
    img_in = nc.dram_tensor("img_in", (rows, F), f32, kind="ExternalInput")
    box_in = nc.dram_tensor("box_in", (brows, 8), f32, kind="ExternalInput")
    img_out = nc.dram_tensor("img_out", (rows, F), f32, kind="ExternalOutput")
    box_out = nc.dram_tensor("box_out", (brows, 8), f32, kind="ExternalOutput")

    g = rows // P
    ntiles = g // kg
    assert g % kg == 0
    seg = kg * F

    vin = img_in.rearrange("(g p) f -> p g f", p=P)
    vout = img_out.rearrange("(g p) f -> p g f", p=P)

    with contextlib.ExitStack() as ctx:
        ti = ctx.enter_context(nc.sbuf_tensor("ti", [P, bufs * seg], f32))
        to = ctx.enter_context(nc.sbuf_tensor("to", [P, bufs * seg], f32))
        bi = ctx.enter_context(nc.sbuf_tensor("bi", [brows, 8], f32))
        bo = ctx.enter_context(nc.sbuf_tensor("bo", [brows, 8], f32))
        # one DMA-completion sem lane per buffer slot: concurrent DMAs on a
        # shared sem would interleave their 16 per-engine +1s (racy)
        s_in = [ctx.enter_context(nc.semaphore(f"s_in{b}"))
                for b in range(bufs)]
        s_out = [ctx.enter_context(nc.semaphore(f"s_out{b}"))
                 for b in range(bufs)]
        s_rev = ctx.enter_context(nc.semaphore("s_rev"))
        s_bx = ctx.enter_context(nc.semaphore("s_bx"))
        s_bxd = ctx.enter_context(nc.semaphore("s_bxd"))
        s_bout = ctx.enter_context(nc.semaphore("s_bout"))

        # ---- boxes: ACT DMA in -> DVE math -> ACT DMA out.  The ACT
        # (out) ring is idle at kernel start, so the box transfer never
        # delays the image in-stream on SP; no gpsimd / PE instructions
        # anywhere ----
        nc.scalar.dma_start(bi[:, :], box_in[:, :]).then_inc(s_bx, 16)
        nc.vector.wait_ge(s_bx, 16)
        nc.vector.tensor_scalar(
            out=bo[:, 0:8:2], in0=bi[:, 0:8:2],
            scalar1=-1.0, scalar2=IMG_SIZE,
            op0=mybir.AluOpType.mult, op1=mybir.AluOpType.add)
        nc.vector.tensor_copy(bo[:, 1:8:2], bi[:, 1:8:2]).then_inc(s_bxd, 1)
        # (box out-DMA deferred to the end of the ACT stream — its wait
        # must not stall the early image in-triggers on the ACT ring)

        # ---- image pipeline ----
        # Tile specs (row-group, col0, col1, in_engine, out_engine).
        # First and last row-groups are split into quarter-rows: a small
        # first tile gets the stream's first byte flowing sooner, small
        # last tiles shorten the pipeline drain chain (after the final
        # in-DMA byte the stream is extended only by copy+out of the
        # final tiles).  During fill the ACT ring is idle and during
        # drain the SP ring is idle, so the edge quarters alternate
        # rings to overlap trigger/descriptor-generation latency.
        assert kg == 1 and ntiles >= 4
        half = (F // 2) // C * C       # 960, pixel-aligned
        quart = (F // 4) // C * C      # 480
        sp, act = nc.sync, nc.scalar
        specs = []
        for q in range(4):             # first group: quarters, rings alt
            specs.append((0, q * quart, (q + 1) * quart,
                          sp if q % 2 == 0 else act, act))
        specs += [(g, 0, F, sp, act) for g in range(1, ntiles - 2)]
        g = ntiles - 2
        specs += [(g, 0, half, sp, act), (g, half, F, sp, act)]
        g = ntiles - 1
        for q in range(4):             # last group: quarters, rings alt
            specs.append((g, q * quart, (q + 1) * quart,
                          sp, act if q % 2 == 0 else sp))

        # Emit the first-group in-triggers before anything else so both
        # rings start fetching immediately; defer SP-routed out-triggers
        # to after every in-trigger so they never gate the in-stream.
        def emit_in(t):
            g, f0, f1, e_in, _ = specs[t]
            b = t % bufs
            if t >= bufs:
                e_in.wait_ge(s_rev, t - bufs + 1)
            e_in.dma_start(
                ti[:, b * seg:b * seg + (f1 - f0)], vin[:, g, f0:f1]
            ).then_inc(s_in[b], 16)

        def emit_out(t):
            g, f0, f1, _, e_out = specs[t]
            b = t % bufs
            e_out.wait_ge(s_rev, t + 1)
            e_out.dma_start(
                vout[:, g, F - f1:F - f0],
                to[:, b * seg:b * seg + (f1 - f0)]
            ).then_inc(s_out[b], 16)

        n_head = 4
        for t in range(n_head):
            emit_in(t)
        deferred_outs = []
        for t in range(len(specs)):
            if t >= n_head:
                emit_in(t)
            b = t % bufs
            k = t // bufs          # how many times slot b has been used
            sl = b * seg
            w = specs[t][2] - specs[t][1]
            # DVE: reversal copy (piece reversed internally; its mirrored
            # position within the row is handled by the out-DMA offset)
            nc.vector.wait_ge(s_in[b], 16 * (k + 1))
            if t >= bufs:
                nc.vector.wait_ge(s_out[b], 16 * k)
            seg_i = ti[:, sl:sl + w].rearrange("p (w c) -> p w c", c=C)
            seg_o = to[:, sl:sl + w].rearrange("p (w c) -> p w c", c=C)
            nc.vector.tensor_copy(
                seg_o, seg_i[:, ::-1, :]).then_inc(s_rev, 1)
            if specs[t][4] is act:
                emit_out(t)
            else:
                deferred_outs.append(t)
        for t in deferred_outs:
            emit_out(t)
        nc.scalar.wait_ge(s_bxd, 1)
        nc.scalar.dma_start(box_out[:, :], bo[:, :]).then_inc(s_bout, 16)
        # make sure every output byte has landed before the NEFF ends;
        # spread the lane waits across engines so no single engine
        # serializes them ahead of the end-of-program barrier
        nc.vector.wait_ge(s_bout, 16)
        ntot = len(specs)
        waiters = [nc.sync, nc.vector, nc.scalar]
        for b in range(bufs):
            uses = (ntot - b + bufs - 1) // bufs
            if uses:
                waiters[b % 3].wait_ge(s_out[b], 16 * uses)
    nc.finalize()
    return nc


def kernel(images, boxes, distances, classes):
    images = np.ascontiguousarray(np.asarray(images, dtype=np.float32))
    boxes = np.ascontiguousarray(np.asarray(boxes, dtype=np.float32))
    distances = np.asarray(distances)
    classes = np.asarray(classes)

    if "nc" not in _cache:
        _cache["nc"] = _build_raw()
    nc = _cache["nc"]

    img_sh = images.reshape(NCORES, ROWS, F)
    box_sh = boxes.reshape(NCORES, BROWS, 8)
    in_maps = [
        {"img_in": img_sh[c], "box_in": box_sh[c]} for c in range(NCORES)
    ]
    res = run_bass_kernel_spmd(nc, in_maps, core_ids=list(range(NCORES)))
    _cache["last_results"] = res

    imgs = np.concatenate(
        [res.results[c]["img_out"].reshape(NPER, H, W, C)
         for c in range(NCORES)], axis=0)
    bxs = np.concatenate(
        [res.results[c]["box_out"].reshape(NPER, 50, 4)
         for c in range(NCORES)], axis=0)
    return imgs, bxs, distances, classes
